# revision 1
# baseline (speedup 1.0000x reference)
"""Trainium2 Bass kernel for nn_CrossAttentionFusion.

Problem (hardcoded shapes): B=2, C1=64, C2=256, D=256, NH=8, HD=32, H=W=64,
n = H*W = 4096 tokens per batch image.

    xl = F_lidar tokens (B, n, C1); xc = F_cam tokens (B, n, C2)
    Q = xl@Wq^T, K = xc@Wk^T, V = xc@Wv^T  (per-head HD=32)
    attn = softmax(QK^T/sqrt(HD)); out = attn@V
    x = LN1(xl@Wres^T + out@Wo^T); x = LN2(x + FFN(x)); return (B, D, H, W)

Sharding: 8 cores, zero collectives. Core i handles batch b=i//4 and the
1024-token q-slice (i%4). K/V for the whole image are recomputed per core.

v2 design notes (vs the fp32r baseline):
  * The whole attention path runs bf16 matmuls (host pre-casts weights and
    activations); scores accumulate fp32 in PSUM.
  * exp is the throughput limiter (33.5M elements/core, ScalarE-only at
    128 lanes x 1.2GHz). Split it across two engines: the qc=0 chain uses
    ScalarE Exp; the qc=1 chain computes exp on VectorE with a Schraudolph
    int16 trick: bf16_bits(exp(x)) ~= int16(A*x + B), one tensor_scalar op
    (PSUM fp32 in -> int16 view of a bf16 tile out). Max elementwise error
    ~3%; softmax normalization (denominator uses the same approx values)
    cancels most of it; simulated end-to-end rel err 2.2e-3.
  * HAM: the baseline ran the entire attention phase PE-throttled (K=4/8)
    because ~800ns PE gaps every period prevented the 3.4us fully-busy
    window needed to unthrottle. With both exps off the critical path and
    3-deep shared score-buffer rotation (PSUM: 3x2 banks scores + 2 banks
    merged AV/denominator accumulators), the PE stays ~95% busy.
  * Denominators: all-ones matmuls col-packed with AV write into the same
    PSUM bank at mirrored 32-row groups (4 concurrent 32-col matmuls).
  * PSUM->SBUF copy-outs are routed to ScalarE where the DVE is busy.
"""

import numpy as np

B, C1, C2, D, NH, H, W = 2, 64, 256, 256, 8, 64, 64
HD = D // NH                 # 32
N_TOK = H * W                # 4096 tokens per image
N_CORES = 8
CORES_PER_B = N_CORES // B   # 4
NQ = N_TOK // CORES_PER_B    # 1024 q tokens per core
EPS = 1e-5
SCALE = HD ** -0.5
KC = N_TOK // 128            # 32 k-chunks
QT_TILES = NQ // 128         # 8 q-tiles of 128
F1 = 4 * D                   # 1024 FFN hidden

# Schraudolph-style exp for bf16-bit-pattern in int16 (trunc semantics):
#   bf16_bits(exp(s*SCALE)) ~= int16(EXP_A*s + EXP_B)
EXP_A = 184.6649652337873 * SCALE
EXP_B = 16250.89

_built = None


def _build():
    from contextlib import ExitStack

    import concourse.mybir as mybir
    import concourse.tile as tile
    from concourse import bacc
    from concourse.masks import make_identity

    F32 = mybir.dt.float32
    BF16 = mybir.dt.bfloat16
    I16 = mybir.dt.int16
    AF = mybir.ActivationFunctionType
    OP = mybir.AluOpType

    nc = bacc.Bacc(trn_type="TRN2", target_bir_lowering=False, debug=False,
                   num_devices=N_CORES)

    # ---- DRAM I/O ----
    xq = nc.dram_tensor("xq", [C1, NQ], BF16, kind="ExternalInput").ap()
    xqf = nc.dram_tensor("xqf", [C1, NQ], F32, kind="ExternalInput").ap()
    xc = nc.dram_tensor("xc", [C2, N_TOK], BF16, kind="ExternalInput").ap()
    wkt = nc.dram_tensor("wkt", [C2, D], BF16, kind="ExternalInput").ap()
    wvt = nc.dram_tensor("wvt", [C2, D], BF16, kind="ExternalInput").ap()
    wqt = nc.dram_tensor("wqt", [C1, D], BF16, kind="ExternalInput").ap()
    wrt = nc.dram_tensor("wrt", [C1, D], F32, kind="ExternalInput").ap()
    wot = nc.dram_tensor("wot", [D, D], BF16, kind="ExternalInput").ap()
    w1t = nc.dram_tensor("w1t", [D, F1], BF16, kind="ExternalInput").ap()
    w2t = nc.dram_tensor("w2t", [F1, D], BF16, kind="ExternalInput").ap()
    g1 = nc.dram_tensor("g1", [D], F32, kind="ExternalInput").ap()
    b1 = nc.dram_tensor("b1", [D], F32, kind="ExternalInput").ap()
    g2 = nc.dram_tensor("g2", [D], F32, kind="ExternalInput").ap()
    b2 = nc.dram_tensor("b2", [D], F32, kind="ExternalInput").ap()
    bf1 = nc.dram_tensor("bf1", [F1], F32, kind="ExternalInput").ap()
    bf2 = nc.dram_tensor("bf2", [D], F32, kind="ExternalInput").ap()
    out = nc.dram_tensor("out", [NQ, D], F32, kind="ExternalOutput").ap()

    with tile.TileContext(nc) as tc, ExitStack() as ctx:
        # ---- persistent SBUF ----
        P = ctx.enter_context(tc.tile_pool(name="persist", bufs=1))

        xq_sb = P.tile([C1, NQ], BF16, name="xq_sb")
        wkt_sb = [P.tile([128, D], BF16, name=f"wkt{c}") for c in range(2)]
        wvt_sb = [P.tile([128, D], BF16, name=f"wvt{c}") for c in range(2)]
        wqt_sb = P.tile([C1, D], BF16, name="wqt_sb")
        wrt_sb = P.tile([C1, D], F32, name="wrt_sb")
        xqf_sb = P.tile([C1, NQ], F32, name="xqf_sb")
        wot_sb = [P.tile([128, D], BF16, name=f"wot{c}") for c in range(2)]
        kt_sb = [P.tile([128, N_TOK], BF16, name=f"kt{g}") for g in range(2)]
        v_sb = P.tile([128, KC, D], BF16, name="v_sb")
        qt_sb = [P.tile([128, NQ], BF16, name=f"qt{g}") for g in range(2)]
        resid_sb = P.tile([128, QT_TILES, D], F32, name="resid_sb")
        attn_sb = [P.tile([128, NQ], BF16, name=f"attn{g}") for g in range(2)]
        rec_bc = [P.tile([128, NQ], F32, name=f"recbc{g}") for g in range(2)]
        ones_bf = P.tile([128, HD], BF16, name="ones_bf")
        ident = P.tile([128, 128], F32, name="ident")
        eps_sb = P.tile([128, 1], F32, name="eps_sb")
        g1_bc = P.tile([128, D], F32, name="g1_bc")
        b1_bc = P.tile([128, D], F32, name="b1_bc")
        g2_bc = P.tile([128, D], F32, name="g2_bc")
        b2_bc = P.tile([128, D], F32, name="b2_bc")
        bf2_bc = P.tile([128, D], F32, name="bf2_bc")

        ones_f32 = P.tile([128, HD], F32, name="ones_f32")
        nc.vector.memset(ones_f32, 1.0)
        nc.vector.tensor_copy(ones_bf, ones_f32)
        nc.vector.memset(eps_sb, EPS)
        make_identity(nc, ident)

        def bcast_row(dst, src_ap, n):
            # (n,) dram -> (128, n) sbuf, replicated on all partitions
            import concourse.bass as bass
            src = bass.AP(tensor=src_ap.tensor, offset=src_ap.offset,
                          ap=[[0, 128]] + src_ap.ap)
            nc.sync.dma_start(dst, src)

        # DMA order matters: the queue is serial, and the first KT matmul
        # needs wkt + the first xc chunk. LN constants come last.
        for c in range(2):
            nc.sync.dma_start(wkt_sb[c], wkt[128 * c:128 * (c + 1), :])

        # =============== Phase A: projections (bf16) ===============
        # KT/QT first in their own PSUM pool (copies on ScalarE). V and
        # resid then run THROUGH the attention score-buffer rotation, so
        # the attention prologue (first scores + exps) can be interleaved
        # mid-V: the first exps complete while the PE is still dense with
        # projection matmuls, and the HAM clock-gate never sees an idle
        # window at the phase transition.
        with tc.tile_pool(name="xc_pool", bufs=1) as XP:
            xc_sb = [XP.tile([128, N_TOK], BF16, name=f"xc{c}")
                     for c in range(2)]
            for ch in range(4):
                cs = slice(1024 * ch, 1024 * (ch + 1))
                for c in range(2):
                    nc.sync.dma_start(xc_sb[c][:, cs],
                                      xc[128 * c:128 * (c + 1), cs])
                if ch == 0:
                    nc.sync.dma_start(wqt_sb, wqt)
                    nc.sync.dma_start(xq_sb, xq)
            for c in range(2):
                nc.sync.dma_start(wvt_sb[c], wvt[128 * c:128 * (c + 1), :])
            nc.sync.dma_start(xqf_sb, xqf)
            nc.sync.dma_start(wrt_sb, wrt)
            for c in range(2):
                nc.sync.dma_start(wot_sb[c], wot[128 * c:128 * (c + 1), :])
            bcast_row(g1_bc, g1, D)
            bcast_row(b1_bc, b1, D)
            bcast_row(g2_bc, g2, D)
            bcast_row(b2_bc, b2, D)
            bcast_row(bf2_bc, bf2, D)

            with tc.tile_pool(name="psA", bufs=4, space="PSUM") as psA:
                # KT[d,k] = sum_c WkT[c,d] * xcT[c,k]
                for g in range(2):
                    for ks in range(8):
                        kp = psA.tile([128, 512], F32, name="kp")
                        for c in range(2):
                            nc.tensor.matmul(
                                kp, wkt_sb[c][:, 128 * g:128 * (g + 1)],
                                xc_sb[c][:, 512 * ks:512 * (ks + 1)],
                                start=(c == 0), stop=(c == 1))
                        nc.scalar.copy(
                            kt_sb[g][:, 512 * ks:512 * (ks + 1)], kp)
                # QT[d,q] = sum_c WqT[c,d] * xqT[c,q]
                for g in range(2):
                    for qs in range(NQ // 512):
                        qp = psA.tile([128, 512], F32, name="kp")
                        nc.tensor.matmul(
                            qp, wqt_sb[:, 128 * g:128 * (g + 1)],
                            xq_sb[:, 512 * qs:512 * (qs + 1)],
                            start=True, stop=True)
                        nc.scalar.copy(
                            qt_sb[g][:, 512 * qs:512 * (qs + 1)], qp)

            # ========= Phase B: V/resid tail + attention =========
            with tc.tile_pool(name="scps", bufs=3, space="PSUM") as scps, \
                 tc.tile_pool(name="avps", bufs=1, space="PSUM") as avps, \
                 tc.tile_pool(name="epool", bufs=12) as epool:
                def hp_params(hp):
                    hA, hB = 2 * hp, 2 * hp + 1
                    pA, pB = 32 * (hA % 4), 32 * (hB % 4)
                    return (hA, hB, hp // 2, pA, pB,
                            (pA + 64) % 128, (pB + 64) % 128)

                scs, es, avs_of = {}, {}, {}

                def emit_scores(hp, kc, again=False):
                    hA, hB, g, pA, pB, oA, oB = hp_params(hp)
                    ks = slice(128 * kc, 128 * (kc + 1))
                    for qc in range(2):
                        qs = slice(512 * qc, 512 * (qc + 1))
                        if again:
                            sc = scs[(hp, kc, qc)]
                        else:
                            sc = scps.tile([128, 1024], F32, name="sc")
                            scs[(hp, kc, qc)] = sc
                        nc.tensor.matmul(
                            sc[:, 0:512], kt_sb[g][pA:pA + 32, ks],
                            qt_sb[g][pA:pA + 32, qs],
                            start=True, stop=True, tile_position=(pA, 0))
                        nc.tensor.matmul(
                            sc[:, 512:1024], kt_sb[g][pB:pB + 32, ks],
                            qt_sb[g][pB:pB + 32, qs],
                            start=True, stop=True, tile_position=(pB, 0))

                def emit_exps(hp, kc):
                    for qc in range(2):
                        sc = scs.pop((hp, kc, qc))
                        e = epool.tile([128, 1024], BF16, name="e")
                        if qc == 0:
                            nc.scalar.activation(e, sc, AF.Exp, scale=SCALE)
                        else:
                            nc.vector.tensor_scalar(
                                out=e.bitcast(I16), in0=sc,
                                scalar1=EXP_A, scalar2=EXP_B,
                                op0=OP.mult, op1=OP.add)
                        es[(hp, kc, qc)] = e

                def emit_avs(hp, kc):
                    hA, hB, g, pA, pB, oA, oB = hp_params(hp)
                    if kc == 0:
                        avs_of[hp] = [
                            avps.tile([128, 512], F32, name=f"av{qc}")
                            for qc in range(2)]
                    avs = avs_of[hp]
                    st, sp = (kc == 0), (kc == KC - 1)
                    for qc in range(2):
                        av = avs[qc]
                        e = es.pop((hp, kc, qc))
                        nc.tensor.matmul(
                            av[pA:pA + 32, :],
                            v_sb[:, kc, HD * hA:HD * hA + HD],
                            e[:, 0:512], start=st, stop=sp,
                            tile_position=(0, pA), skip_group_check=True)
                        nc.tensor.matmul(
                            av[pB:pB + 32, :],
                            v_sb[:, kc, HD * hB:HD * hB + HD],
                            e[:, 512:1024], start=st, stop=sp,
                            tile_position=(0, pB), skip_group_check=True)
                        nc.tensor.matmul(
                            av[oA:oA + 32, :], ones_bf, e[:, 0:512],
                            start=st, stop=sp, tile_position=(0, oA),
                            skip_group_check=True)
                        nc.tensor.matmul(
                            av[oB:oB + 32, :], ones_bf, e[:, 512:1024],
                            start=st, stop=sp, tile_position=(0, oB),
                            skip_group_check=True)
                    if sp:
                        # drain the hp's accumulators. Deprioritized ~3
                        # pipeline bodies so the scheduler keeps the exp
                        # cadence ahead of them; the deep e-tile pool
                        # absorbs the deferred AV bank handoff.
                        ctx_hp = tc.high_priority(offset=-60)
                        ctx_hp.__enter__()
                        for qc in range(2):
                            qs = slice(512 * qc, 512 * (qc + 1))
                            if qc == 0:
                                nc.scalar.copy(
                                    attn_sb[g][pA:pA + 64, qs],
                                    avs[qc][pA:pA + 64, :])
                                nc.vector.tensor_copy(
                                    rec_bc[g][pA:pA + 64, qs],
                                    avs[qc][oA:oA + 64, :])
                            else:
                                nc.vector.tensor_copy(
                                    attn_sb[g][pA:pA + 64, qs],
                                    avs[qc][pA:pA + 64, :])
                                nc.scalar.copy(
                                    rec_bc[g][pA:pA + 64, qs],
                                    avs[qc][oA:oA + 64, :])
                        ctx_hp.__exit__(None, None, None)

                def view3(t):
                    import concourse.bass as bass
                    return bass.AP(tensor=t.tensor, offset=t.offset,
                                   ap=[t.ap[0], [D, 4], [1, D]])

                def emit_v(i4):
                    # V[k,d] for 4 k-chunks through one score-rotation
                    # tile; one [128,1024] copy-out on DVE.
                    vp = scps.tile([128, 1024], F32, name="sc")
                    vpv = view3(vp)
                    for j in range(4):
                        kt_i = i4 * 4 + j
                        for c in range(2):
                            nc.tensor.matmul(
                                vpv[:, j, :],
                                xc_sb[c][:, 128 * kt_i:128 * (kt_i + 1)],
                                wvt_sb[c], start=(c == 0), stop=(c == 1))
                    nc.vector.tensor_copy(
                        v_sb[:, 4 * i4:4 * (i4 + 1), :], vpv)

                def emit_resid(i4):
                    rp = scps.tile([128, 1024], F32, name="sc")
                    rpv = view3(rp)
                    for j in range(4):
                        qt_i = i4 * 4 + j
                        nc.tensor.matmul(
                            rpv[:, j, :],
                            xqf_sb[:, 128 * qt_i:128 * (qt_i + 1)],
                            wrt_sb, start=True, stop=True)
                    nc.vector.tensor_copy(
                        resid_sb[:, 4 * i4:4 * (i4 + 1), :], rpv)

                units = [(hp, kc) for hp in range(4) for kc in range(KC)]
                NU, LAG = len(units), 0
                # V head, then the attention prologue (its exps run on
                # ScalarE/VectorE while the PE chews the V tail), then
                # the rest of V + resid, then the pipeline: scores and
                # exps for unit i+1, AV for unit i-LAG. The deep e-tile
                # buffer decouples AV-side stalls (hp-boundary copies)
                # from the score/exp cadence.
                for i4 in range(5):
                    emit_v(i4)
                emit_scores(0, 0)
                emit_exps(0, 0)
                for i4 in range(5, 8):
                    emit_v(i4)
                emit_resid(0)
                emit_resid(1)
                for i in range(NU + LAG):
                    if i + 1 < NU:
                        emit_scores(*units[i + 1])
                        emit_exps(*units[i + 1])
                    if i >= LAG:
                        emit_avs(*units[i - LAG])

        # normalize attn_out^T by 1/sumexp (denominators already swapped
        # into alignment by the copy-outs above). The reciprocal runs on
        # ScalarE as exp(-ln(x)) — both functions live in one ACT table
        # set, and this avoids the DVE's ~8us iterative-divide reciprocal.
        for g in range(2):
            nc.scalar.activation(rec_bc[g], rec_bc[g], AF.Ln)
        for g in range(2):
            nc.scalar.activation(rec_bc[g], rec_bc[g], AF.Exp, scale=-1.0)
        for g in range(2):
            nc.vector.tensor_mul(attn_sb[g], attn_sb[g], rec_bc[g])

        # =============== Phase C: Wo + LN1 + transpose ===============
        with tc.tile_pool(name="post", bufs=1) as POST, \
             tc.tile_pool(name="psC", bufs=2, space="PSUM") as psC, \
             tc.tile_pool(name="tpps", bufs=2, space="PSUM") as tpps, \
             tc.tile_pool(name="lnp", bufs=4) as lnp:
            w1t_sb = [POST.tile([128, F1], BF16, name=f"w1t{c}")
                      for c in range(2)]
            w2t_sb = POST.tile([128, 8, D], BF16, name="w2t_sb")
            x1_sb = POST.tile([128, QT_TILES, D], F32, name="x1_sb")
            x1t_sb = [POST.tile([128, NQ], BF16, name=f"x1t{g}")
                      for g in range(2)]
            hdn_sb = POST.tile([128, 8, NQ], BF16, name="hdn_sb")
            bf1_col = POST.tile([128, 8], F32, name="bf1_col")
            for c in range(2):
                nc.sync.dma_start(w1t_sb[c], w1t[128 * c:128 * (c + 1), :])
            nc.sync.dma_start(
                w2t_sb, w2t.rearrange("(a p) d -> p a d", p=128))
            nc.sync.dma_start(bf1_col, bf1.rearrange("(a p) -> p a", p=128))
            import concourse.bass as bass

            def rep4(t):
                # [128, D] -> [128, 4, D] AP, 0-stride middle dim
                return bass.AP(tensor=t.tensor, offset=t.offset,
                               ap=[t.ap[0], [0, 4], t.ap[1]])

            def ln_tile(src_psum, add_sb, dst, use_bf2):
                # dst = (x - mean(x)) / sqrt(var(x) + eps), where
                # x = src_psum + add_sb (+ bf2).
                xp = lnp.tile([128, D], F32, name="xp")
                nc.vector.tensor_add(xp, src_psum, add_sb)
                if use_bf2:
                    nc.vector.tensor_add(xp, xp, bf2_bc)
                stats = lnp.tile([128, 6], F32, name="stats")
                nc.vector.bn_stats(out=stats, in_=xp)
                mv = lnp.tile([128, 2], F32, name="mv")
                nc.vector.bn_aggr(out=mv, in_=stats)
                rstd = lnp.tile([128, 1], F32, name="rstd")
                nc.scalar.activation(rstd, mv[:, 1:2], AF.Sqrt, bias=eps_sb)
                nc.vector.reciprocal(rstd, rstd)
                nc.vector.tensor_scalar(
                    out=dst, in0=xp, scalar1=mv[:, 0:1], scalar2=rstd,
                    op0=OP.subtract, op1=OP.mult)

            for qt_i in range(QT_TILES):
                ts = slice(128 * qt_i, 128 * (qt_i + 1))
                pp = psC.tile([128, D], F32, name="pp")
                for g in range(2):
                    nc.tensor.matmul(pp, attn_sb[g][:, ts], wot_sb[g],
                                     start=(g == 0), stop=(g == 1))
                ln_tile(pp, resid_sb[:, qt_i, :], x1_sb[:, qt_i, :], False)
            # batched LN1 affine (two halves, pipelined), then transposes
            for h in range(2):
                hs = slice(4 * h, 4 * (h + 1))
                nc.vector.tensor_mul(x1_sb[:, hs, :], x1_sb[:, hs, :],
                                     rep4(g1_bc))
                nc.vector.tensor_add(x1_sb[:, hs, :], x1_sb[:, hs, :],
                                     rep4(b1_bc))
            for qt_i in range(QT_TILES):
                ts = slice(128 * qt_i, 128 * (qt_i + 1))
                for dc in range(2):
                    tp = tpps.tile([128, 128], F32, name="tp")
                    nc.tensor.transpose(
                        tp, x1_sb[:, qt_i, 128 * dc:128 * (dc + 1)], ident)
                    if dc == 0:
                        nc.scalar.copy(x1t_sb[dc][:, ts], tp)
                    else:
                        nc.vector.tensor_copy(x1t_sb[dc][:, ts], tp)

            # =============== Phase D: FFN + LN2 ===============
            ps1 = psC
            ps2 = tpps
            lnp2 = lnp
            # hdn^T[f,q] = relu(sum_d W1T[d,f] x1T[d,q] + bf1[f])
            # (relu+bias on ScalarE)
            for fc in range(8):
                for qc in range(NQ // 512):
                    qs = slice(512 * qc, 512 * (qc + 1))
                    hp_ = ps1.tile([128, 512], F32, name="hp_")
                    for dc in range(2):
                        nc.tensor.matmul(
                            hp_, w1t_sb[dc][:, 128 * fc:128 * (fc + 1)],
                            x1t_sb[dc][:, qs], start=(dc == 0), stop=(dc == 1))
                    nc.scalar.activation(
                        hdn_sb[:, fc, qs], hp_, AF.Relu,
                        bias=bf1_col[:, fc:fc + 1])
            # ffn[q,d] = sum_f hdnT[f,q] W2T[f,d]; x2 = LN2(x1+ffn+bf2)
            x2_sb = POST.tile([128, QT_TILES, D], F32, name="x2_sb")
            for qt_i in range(QT_TILES):
                ts = slice(128 * qt_i, 128 * (qt_i + 1))
                fp = ps2.tile([128, D], F32, name="fp")
                for fc in range(8):
                    nc.tensor.matmul(fp, hdn_sb[:, fc, ts], w2t_sb[:, fc, :],
                                     start=(fc == 0), stop=(fc == 7))
                ln_tile(fp, x1_sb[:, qt_i, :], x2_sb[:, qt_i, :], True)
            # batched LN2 affine (two halves), then store
            for h in range(2):
                hs = slice(4 * h, 4 * (h + 1))
                nc.vector.tensor_mul(x2_sb[:, hs, :], x2_sb[:, hs, :],
                                     rep4(g2_bc))
                nc.vector.tensor_add(x2_sb[:, hs, :], x2_sb[:, hs, :],
                                     rep4(b2_bc))
            for qt_i in range(QT_TILES):
                ts = slice(128 * qt_i, 128 * (qt_i + 1))
                nc.sync.dma_start(out[ts, :], x2_sb[:, qt_i, :])

    nc.compile()
    return nc


def _get_nc():
    global _built
    if _built is None:
        _built = _build()
    return _built


def _make_in_maps(inputs):
    import ml_dtypes
    f32 = np.float32
    bf16 = ml_dtypes.bfloat16
    F_lidar = np.ascontiguousarray(inputs["F_lidar"], dtype=f32)
    F_cam = np.ascontiguousarray(inputs["F_cam"], dtype=f32)
    common = {
        "wkt": np.ascontiguousarray(np.asarray(inputs["Wk"]).T.astype(bf16)),
        "wvt": np.ascontiguousarray(np.asarray(inputs["Wv"]).T.astype(bf16)),
        "wqt": np.ascontiguousarray(np.asarray(inputs["Wq"]).T.astype(bf16)),
        "wrt": np.ascontiguousarray(inputs["Wres"].T, f32),
        "wot": np.ascontiguousarray(np.asarray(inputs["Wo"]).T.astype(bf16)),
        "w1t": np.ascontiguousarray(np.asarray(inputs["W1"]).T.astype(bf16)),
        "w2t": np.ascontiguousarray(np.asarray(inputs["W2"]).T.astype(bf16)),
        "g1": np.asarray(inputs["g1"], f32), "b1": np.asarray(inputs["b1"], f32),
        "g2": np.asarray(inputs["g2"], f32), "b2": np.asarray(inputs["b2"], f32),
        "bf1": np.asarray(inputs["bf1"], f32),
        "bf2": np.asarray(inputs["bf2"], f32),
    }
    in_maps = []
    for c in range(N_CORES):
        b, s = c // CORES_PER_B, (c % CORES_PER_B) * NQ
        m = dict(common)
        xq_f = np.ascontiguousarray(
            F_lidar[b].reshape(C1, N_TOK)[:, s:s + NQ])
        m["xq"] = xq_f.astype(bf16)
        m["xqf"] = xq_f
        m["xc"] = np.ascontiguousarray(
            F_cam[b].reshape(C2, N_TOK)).astype(bf16)
        in_maps.append(m)
    return in_maps


def kernel(**inputs):
    from concourse.bass_utils import run_bass_kernel_spmd

    nc = _get_nc()
    in_maps = _make_in_maps(inputs)
    res = run_bass_kernel_spmd(nc, in_maps, list(range(N_CORES)))
    out = np.empty((B, D, N_TOK), dtype=np.float32)
    for c in range(N_CORES):
        b, s = c // CORES_PER_B, (c % CORES_PER_B) * NQ
        out[b, :, s:s + NQ] = res.results[c]["out"].T
    return out.reshape(B, D, H, W)

